# revision 51
# baseline (speedup 1.0000x reference)
"""Trainium2 Bass kernel v7 for DirectedGraphConv.

Math (per batch b, node n):
    out = feature + einsum("bni,doi->bno", feature, weights) + bias[graph].sum(axis=2)

Identities:
  * einsum sums over directions d and input dim i:  out_mm = F @ (W0+W1+I)^T
    (+feature folds in as +I, added to the direction-sum on device).
  * bias[graph].sum(axis=2) = Count @ bias.  Counts come from 16 histogram
    matmuls over 16 linearly-independent "plane" functions of the labels:
    11 is_equal indicators (DVE), 4 ReLU ramps relu(g - a), a=11.5..14.5
    (ACT), and an all-ones plane (memset).  The change of basis back to
    per-label counts is an exact small-integer matrix A folded into the
    matmul selector stationaries (built on-chip).

Final design (v16; ~24.8us vs the 31.5us v6 baseline):
  * All inputs ride ONE packed bf16 HBM tensor, per-partition layout
    [g | cb4 | ftc0 | wtc0 | ... | ftc3 | wtc3] (g pre-cast to bf16 on the
    host — pure cast), moved by 5 DMAs split across BOTH HWDGE rings
    (sync: g, [cb4|c0], c2; scalar: c1, c3).  Dual-ring streaming lifts
    input bandwidth from ~230 to ~300+ GB/s; the g piece goes first since
    it feeds the longest dependency chain (planes -> counts -> bias).
  * Histogram matmuls are 4x column-tiled (M=16 output rows only): planes
    land in disjoint 32-partition PSUM strips via tile_position, fired
    individually in plane-readiness order (stops = last one per strip).
    The strip reduction is folded into the bias matmul: cb4 replicates
    bias into all four strips (host-padded zeros), so the 128-partition
    contraction sums the strips for free.
  * The DVE delta stream runs uninterrupted (it gates the cnt->cntT->bias
    chain); Wsum = W0+W1+I adds follow in DMA-arrival order.  gpsimd does
    ONLY memset/affine const building: its elementwise ops share an SBUF
    port with the DVE under an exclusive lock (measured ~25x slowdown).
  * PE warm-up fillers + bridge fillers keep HAM at K=8/8 (cold matmuls
    are 2x slower); the dense tail is bias -> mains c0..c2 -> mains c3
    closers per bank; paired output DMAs (banks 01 sync, 23 scalar).

Device does all arithmetic.  Host only reshapes/transposes/casts/pads
inputs and upcasts the bf16 output.
"""

import numpy as np
import ml_dtypes

BF16 = ml_dtypes.bfloat16

B, N, D = 32, 128, 512
DIR = 2
L = 16
NC = 8
BPC = B // NC  # 4
BN = BPC * N  # 512
P = 128
KC = D // P  # 4
NDELTA = 11  # is_equal planes (labels 0..10)
NRAMP = 4  # relu ramp planes
FILLERS = 6

# packed bf16 tensor field offsets (in bf16 elements, per partition):
# [ g(bf16) | cb4 | wtc0 | ftc0 | wtc1 | ftc1 | wtc2 | ftc2 | wtc3 | ftc3 ]
# wt before ft within each chunk: the Wsum adds (on the critical path to
# the mains matmuls) wait on the wt piece only.
G_OFF = 0
CB4_OFF = BN  # 512
HEAD_LEN = BN + D  # 1024
CHUNK_LEN = BN + DIR * D  # wtc (1024) + ftc (512) = 1536
PK_LEN = HEAD_LEN + KC * CHUNK_LEN  # 7168


def _ft_off(c):
    return HEAD_LEN + c * CHUNK_LEN + DIR * D


def _wt_off(c, d):
    return HEAD_LEN + c * CHUNK_LEN + d * D


# DMA split points (bf16 elements): g | [cb4|wt0] | ft0 | wt1 | ft1 | ...
# Small pieces complete earlier, releasing their consumers sooner.
_PK_SPLITS = [0, BN, HEAD_LEN + DIR * D]
for _c in range(1, KC):
    _PK_SPLITS += [HEAD_LEN + _c * CHUNK_LEN, HEAD_LEN + _c * CHUNK_LEN + DIR * D]
_PK_SPLITS.append(PK_LEN)
# ring per piece: sync: g, [cb4|wt0], ft0, wt2, ft2; scalar: wt1, ft1, wt3, ft3
_PK_ENGS = ("sync", "sync", "sync", "scalar", "scalar",
            "sync", "sync", "scalar", "scalar")

# plane ids: 0..10 = delta_l, 11..14 = ramp(11.5+i), 15 = ones
# cnt matmul firing order (by expected plane readiness: ones via memset,
# d0-d10 on DVE uninterrupted, ramps r11-r14 on ACT); strip = index % 4
CNT_ORDER = [15, 0, 1, 2, 3, 4, 5, 6, 7, 11, 12, 8, 13, 9, 10, 14]


def _amatrix():
    xs = np.arange(L)
    planes = [(xs == l).astype(np.float64) for l in range(NDELTA)]
    for i in range(NRAMP):
        planes.append(np.maximum(xs - (NDELTA + 0.5 + i), 0.0))
    planes.append(np.ones(L))
    M = np.stack(planes)  # [16 planes, 16 labels]
    A = np.linalg.inv(M)  # counts = A @ S
    assert np.abs(A - np.round(A)).max() < 1e-9
    return np.round(A)  # A[j, k]: weight of plane k into count row j


_prog_cache: dict = {}


def _build():
    import concourse.bass as bass  # noqa: F401
    import concourse.mybir as mybir
    import concourse.tile as tile
    from concourse import bacc
    from concourse.masks import make_identity

    f32 = mybir.dt.float32
    bf16 = mybir.dt.bfloat16
    u8 = mybir.dt.uint8

    nc = bacc.Bacc("TRN2", target_bir_lowering=False, debug=False, num_devices=NC)

    pk = nc.dram_tensor("pk", [P, PK_LEN], bf16, kind="ExternalInput").ap()
    # [n, b, d] layout: output DMAs are then descriptor-contiguous per
    # partition (no rearrange); the host transposes back for free
    out = nc.dram_tensor("out", [N, BPC, D], bf16, kind="ExternalOutput").ap()

    A = _amatrix()

    with tile.TileContext(nc) as tc:
        with (
            tc.tile_pool(name="work", bufs=1) as wpool,
            tc.tile_pool(name="psum", bufs=1, space="PSUM") as ppool,
        ):
            # ---- DMAs first: g (small, feeds the longest chain), then the
            # wt/ft pieces split across both HWDGE rings so the two queues
            # stream concurrently and each consumer waits only its piece
            pk_sb = wpool.tile([P, PK_LEN], bf16)
            for i, ename in enumerate(_PK_ENGS):
                eng = nc.sync if ename == "sync" else nc.scalar
                s, e = _PK_SPLITS[i], _PK_SPLITS[i + 1]
                eng.dma_start(out=pk_sb[:, s:e], in_=pk[:, s:e])

            # ---- on-chip constants (gpsimd) ----
            # esel[m, k, j] = A[j, k] for j < 16, 0 for j in 16..31.
            esel = wpool.tile([P, L, 32], bf16)
            nc.gpsimd.memset(esel, 0.0)
            esel_d = esel[:, 0:NDELTA, 0:L]
            # fill 1.0 on the delta diagonal (j == k)
            nc.gpsimd.affine_select(
                out=esel_d,
                in_=esel_d,
                compare_op=mybir.AluOpType.not_equal,
                fill=1.0,
                base=0,
                pattern=[[1, NDELTA], [-1, L]],
                channel_multiplier=0,
            )
            # delta planes also contribute -1 to count row 11
            nc.gpsimd.affine_select(
                out=esel_d,
                in_=esel_d,
                compare_op=mybir.AluOpType.not_equal,
                fill=-1.0,
                base=-NDELTA,
                pattern=[[0, NDELTA], [1, L]],
                channel_multiplier=0,
            )
            # ones plane column (needed by round 0)
            nc.gpsimd.memset(esel[:, 15, 11:12], float(A[11, 15]))
            # ones plane data + relu bias constants + identity
            planes = wpool.tile([P, L, BN], bf16)
            nc.gpsimd.memset(planes[:, 15, :], 1.0)
            rpb = wpool.tile([P, NRAMP], f32)
            for i in range(NRAMP):
                nc.gpsimd.memset(rpb[:, i : i + 1], -(NDELTA + 0.5 + i))
            ident = wpool.tile([P, P], bf16)
            make_identity(nc, ident)
            # ramp-plane selector columns (needed by round 3)
            for k in range(NDELTA, L - 1):
                for j in range(NDELTA, L):
                    v = float(A[j, k])
                    if v != 0.0:
                        nc.gpsimd.memset(esel[:, k, j : j + 1], v)

            # ---- ACT: table preload, then ramps as soon as G_bf exists ----
            act_warm = wpool.tile([P, 2], f32)
            nc.scalar.copy(out=act_warm[:, 0:1], in_=act_warm[:, 1:2])

            # ---- DVE stream (g arrives pre-cast to bf16 in the pk head) ----
            dummy = wpool.tile([P, BN], bf16)
            nc.vector.memset(dummy, 1.0)
            G_bf = pk_sb[:, G_OFF : G_OFF + BN]

            # ACT ramps (program order on scalar engine; wait on G_bf)
            for i in range(NRAMP):
                nc.scalar.activation(
                    out=planes[:, NDELTA + i, :],
                    in_=G_bf,
                    func=mybir.ActivationFunctionType.Relu,
                    bias=rpb[:, i : i + 1],
                    scale=1.0,
                )

            Wsum = wpool.tile([P, KC, D], bf16)

            def _wsum(c):
                nc.vector.tensor_tensor(
                    out=Wsum[:, c, :],
                    in0=pk_sb[:, _wt_off(c, 0) : _wt_off(c, 0) + D],
                    in1=pk_sb[:, _wt_off(c, 1) : _wt_off(c, 1) + D],
                    op=mybir.AluOpType.add,
                )
                sl = slice(c * P, (c + 1) * P)
                nc.vector.tensor_tensor(
                    out=Wsum[:, c, sl],
                    in0=Wsum[:, c, sl],
                    in1=ident,
                    op=mybir.AluOpType.add,
                )

            def _delta(l, eng):
                eng.tensor_scalar(
                    out=planes[:, l, :],
                    in0=G_bf,
                    scalar1=float(l),
                    scalar2=None,
                    op0=mybir.AluOpType.is_equal,
                )

            # DVE: the delta stream runs UNINTERRUPTED (it feeds the long
            # cnt->cntT->bias chain); Wsum adds follow in DMA arrival order:
            # c0 (sync, after g), c1 (scalar head), c2 (sync), c3 (scalar).
            # (gpsimd must NOT run elementwise here: its DVE-shared SBUF port
            # lock makes both engines ~25x slower — measured)
            for l in range(NDELTA):
                _delta(l, nc.vector)
            # arrival order: wt1 (scalar head) ~ wt0 (sync), then wt3, wt2
            for c in (1, 0, 3, 2):
                _wsum(c)

            # ---- PE stream ----
            psum_warm = ppool.tile([P, BN], f32, tag="warm", bufs=1)
            psum_cnt = ppool.tile([P, BN], f32, tag="cnt", bufs=1)
            psum_outs = [
                ppool.tile([P, D], f32, tag=f"out{b}", bufs=1, name=f"psum_out{b}")
                for b in range(BPC)
            ]

            for _ in range(FILLERS):
                nc.tensor.matmul(
                    out=psum_warm, lhsT=dummy[:, 0:P], rhs=dummy,
                    start=True, stop=True,
                )

            def cnt_mm(i):
                k = CNT_ORDER[i]
                s = i % 4
                nc.tensor.matmul(
                    out=psum_cnt[32 * s : 32 * s + 32, :],
                    lhsT=esel[:, k, :],
                    rhs=planes[:, k, :],
                    start=i < 4,
                    stop=i >= 12,
                    tile_position=(0, 32 * s),
                )

            def mains(c, start=False, stop=False):
                fo = _ft_off(c)
                for b in range(BPC):
                    nc.tensor.matmul(
                        out=psum_outs[b],
                        lhsT=pk_sb[:, fo + b * P : fo + (b + 1) * P],
                        rhs=Wsum[:, c, :],
                        start=start,
                        stop=stop,
                    )

            for i in range(16):
                cnt_mm(i)

            # single whole-tile copy: 4 slice copies self-serialize on ACT
            # with ~400ns sem gaps each (measured) and finish later
            cntT = wpool.tile([P, BN], bf16)
            nc.scalar.copy(out=cntT, in_=psum_cnt)

            # bridge filler keeps the PE HAM-warm between the cnt stream and
            # the dense tail
            nc.tensor.matmul(
                out=psum_warm, lhsT=dummy[:, 0:P], rhs=dummy,
                start=True, stop=True,
            )

            # tail: mains c1 (earliest Wsum), bias, mains c0, c3, then c2
            # (last to land) closing each output bank progressively
            cb4 = pk_sb[:, CB4_OFF : CB4_OFF + D]
            out_sb = wpool.tile([P, BPC, D], bf16)
            h = D // 2
            mains(1, start=True)
            mains(0)
            # bias after mains c0: cntT (ready ~14us) can then never stall
            # the dense tail stream
            for b in range(BPC):
                nc.tensor.matmul(
                    out=psum_outs[b],
                    lhsT=cntT[:, b * P : (b + 1) * P],
                    rhs=cb4,
                    start=False,
                    stop=False,
                )
            mains(3)
            fo2 = _ft_off(2)
            for b in range(BPC):
                nc.tensor.matmul(
                    out=psum_outs[b],
                    lhsT=pk_sb[:, fo2 + b * P : fo2 + (b + 1) * P],
                    rhs=Wsum[:, 2, :],
                    start=False,
                    stop=True,
                )
                nc.vector.tensor_copy(out=out_sb[:, b, 0:h], in_=psum_outs[b][:, 0:h])
                nc.scalar.copy(out=out_sb[:, b, h:D], in_=psum_outs[b][:, h:D])
                # two paired output DMAs (fewer ~650ns HWDGE issues): banks
                # 0-1 on the sync ring, banks 2-3 on the scalar ring; the
                # [n, b, d] HBM layout keeps descriptors contiguous
                if b == 1:
                    nc.sync.dma_start(out=out[:, 0:2, :], in_=out_sb[:, 0:2, :])
                elif b == 3:
                    nc.scalar.dma_start(out=out[:, 2:4, :], in_=out_sb[:, 2:4, :])

    nc.compile()
    return nc


def _get_prog():
    if "v7" not in _prog_cache:
        _prog_cache["v7"] = _build()
    return _prog_cache["v7"]


def _shard_inputs(feature, graph, weights, bias):
    f = np.asarray(feature, dtype=np.float32)
    g8 = np.asarray(graph).astype(np.uint8)
    w = np.asarray(weights, dtype=np.float32)
    b16 = np.asarray(bias, dtype=np.float32).astype(BF16)

    # cb4[p] = bias[p % 32] if p % 32 < 16 else 0   (strip-replicated)
    cb4 = np.zeros((P, D), dtype=BF16)
    for s in range(4):
        cb4[32 * s : 32 * s + L] = b16

    # wt[p, c, d, o] = w[d, o, c*128+p]   (replicated across cores)
    wt = np.ascontiguousarray(
        w.transpose(2, 0, 1).reshape(KC, P, DIR, D).transpose(1, 0, 2, 3)
    ).astype(BF16)  # [p, c, d, o]

    in_maps = []
    for core in range(NC):
        sl = slice(core * BPC, (core + 1) * BPC)
        fc = f[sl]  # [BPC, N, D]
        ftc = np.ascontiguousarray(
            fc.transpose(2, 0, 1).reshape(KC, P, BN).transpose(1, 0, 2)
        ).astype(BF16)  # [p, c, bn]
        gc = np.ascontiguousarray(g8[sl].transpose(2, 0, 1).reshape(P, BN))
        pk = np.empty((P, PK_LEN), dtype=BF16)
        pk[:, G_OFF : G_OFF + BN] = gc.astype(BF16)
        pk[:, CB4_OFF : CB4_OFF + D] = cb4
        for c in range(KC):
            pk[:, _ft_off(c) : _ft_off(c) + BN] = ftc[:, c, :]
            pk[:, _wt_off(c, 0) : _wt_off(c, 1) + D] = wt[:, c].reshape(P, DIR * D)
        in_maps.append({"pk": pk})
    return in_maps


def _run(feature, graph, weights, bias, trace=False):
    from concourse.bass_utils import run_bass_kernel_spmd

    in_maps = _shard_inputs(feature, graph, weights, bias)
    nc = _get_prog()
    res = run_bass_kernel_spmd(nc, in_maps, core_ids=list(range(NC)), trace=trace)
    # device output is [n, b, d]; transpose back to [b, n, d] per core
    out = np.concatenate(
        [r["out"].astype(np.float32).transpose(1, 0, 2) for r in res.results],
        axis=0,
    )
    return out, res


def kernel(feature, graph, weights, bias):
    out, _ = _run(feature, graph, weights, bias, trace=False)
    return out


# revision 55
# speedup vs baseline: 1.1728x; 1.1728x over previous
"""Trainium2 Bass kernel v7 for DirectedGraphConv.

Math (per batch b, node n):
    out = feature + einsum("bni,doi->bno", feature, weights) + bias[graph].sum(axis=2)

Identities:
  * einsum sums over directions d and input dim i:  out_mm = F @ (W0+W1+I)^T
    (+feature folds in as +I, added to the direction-sum on device).
  * bias[graph].sum(axis=2) = Count @ bias.  Counts come from 16 histogram
    matmuls over 16 linearly-independent "plane" functions of the labels:
    11 is_equal indicators (DVE), 4 ReLU ramps relu(g - a), a=11.5..14.5
    (ACT), and an all-ones plane (memset).  The change of basis back to
    per-label counts is an exact small-integer matrix A folded into the
    matmul selector stationaries (built on-chip).

Final design (v16; ~24.8us vs the 31.5us v6 baseline):
  * All inputs ride ONE packed bf16 HBM tensor, per-partition layout
    [g | cb4 | ftc0 | wtc0 | ... | ftc3 | wtc3] (g pre-cast to bf16 on the
    host — pure cast), moved by 5 DMAs split across BOTH HWDGE rings
    (sync: g, [cb4|c0], c2; scalar: c1, c3).  Dual-ring streaming lifts
    input bandwidth from ~230 to ~300+ GB/s; the g piece goes first since
    it feeds the longest dependency chain (planes -> counts -> bias).
  * Histogram matmuls are 4x column-tiled (M=16 output rows only): planes
    land in disjoint 32-partition PSUM strips via tile_position, fired
    individually in plane-readiness order (stops = last one per strip).
    The strip reduction is folded into the bias matmul: cb4 replicates
    bias into all four strips (host-padded zeros), so the 128-partition
    contraction sums the strips for free.
  * The DVE delta stream runs uninterrupted (it gates the cnt->cntT->bias
    chain); Wsum = W0+W1+I adds follow in DMA-arrival order.  gpsimd does
    ONLY memset/affine const building: its elementwise ops share an SBUF
    port with the DVE under an exclusive lock (measured ~25x slowdown).
  * PE warm-up fillers + bridge fillers keep HAM at K=8/8 (cold matmuls
    are 2x slower); the dense tail is bias -> mains c0..c2 -> mains c3
    closers per bank; paired output DMAs (banks 01 sync, 23 scalar).

Device does all arithmetic.  Host only reshapes/transposes/casts/pads
inputs and upcasts the bf16 output.
"""

import numpy as np
import ml_dtypes

BF16 = ml_dtypes.bfloat16

B, N, D = 32, 128, 512
DIR = 2
L = 16
NC = 8
BPC = B // NC  # 4
BN = BPC * N  # 512
P = 128
KC = D // P  # 4
NDELTA = 11  # is_equal planes (labels 0..10)
NRAMP = 4  # relu ramp planes
FILLERS = 6

# packed bf16 tensor field offsets (in bf16 elements, per partition):
# [ g(bf16) | cb4 | wtc0 | ftc0 | wtc1 | ftc1 | wtc2 | ftc2 | wtc3 | ftc3 ]
# wt before ft within each chunk: the Wsum adds (on the critical path to
# the mains matmuls) wait on the wt piece only.
G_OFF = 0
CB4_OFF = BN  # 512
HEAD_LEN = BN + D  # 1024
CHUNK_LEN = BN + DIR * D  # wtc (1024) + ftc (512) = 1536
PK_LEN = HEAD_LEN + KC * CHUNK_LEN  # 7168


def _ft_off(c):
    return HEAD_LEN + c * CHUNK_LEN + DIR * D


def _wt_off(c, d):
    return HEAD_LEN + c * CHUNK_LEN + d * D


# DMA split points (bf16 elements): g | [cb4|wt0] | ft0 | wt1 | ft1 | ...
# Small pieces complete earlier, releasing their consumers sooner.
_PK_SPLITS = [0, BN, HEAD_LEN + DIR * D]
for _c in range(1, KC):
    _PK_SPLITS += [HEAD_LEN + _c * CHUNK_LEN, HEAD_LEN + _c * CHUNK_LEN + DIR * D]
_PK_SPLITS.append(PK_LEN)
# ring per piece: sync: g, [cb4|wt0], ft0, wt2, ft2; scalar: wt1, ft1, wt3, ft3
_PK_ENGS = ("sync", "sync", "sync", "scalar", "scalar",
            "sync", "sync", "scalar", "scalar")

# plane ids: 0..10 = delta_l, 11..14 = ramp(11.5+i), 15 = ones
# cnt matmul firing order (by expected plane readiness: ones via memset,
# d0-d10 on DVE uninterrupted, ramps r11-r14 on ACT); strip = index % 4
CNT_ORDER = [15, 0, 1, 2, 3, 4, 5, 6, 7, 11, 12, 8, 13, 9, 10, 14]


def _amatrix():
    xs = np.arange(L)
    planes = [(xs == l).astype(np.float64) for l in range(NDELTA)]
    for i in range(NRAMP):
        planes.append(np.maximum(xs - (NDELTA + 0.5 + i), 0.0))
    planes.append(np.ones(L))
    M = np.stack(planes)  # [16 planes, 16 labels]
    A = np.linalg.inv(M)  # counts = A @ S
    assert np.abs(A - np.round(A)).max() < 1e-9
    return np.round(A)  # A[j, k]: weight of plane k into count row j


_prog_cache: dict = {}


def _build():
    import concourse.bass as bass  # noqa: F401
    import concourse.mybir as mybir
    import concourse.tile as tile
    from concourse import bacc
    from concourse.masks import make_identity

    f32 = mybir.dt.float32
    bf16 = mybir.dt.bfloat16
    u8 = mybir.dt.uint8

    nc = bacc.Bacc("TRN2", target_bir_lowering=False, debug=False, num_devices=NC)

    pk = nc.dram_tensor("pk", [P, PK_LEN], bf16, kind="ExternalInput").ap()
    out = nc.dram_tensor("out", [BPC, N, D], bf16, kind="ExternalOutput").ap()

    A = _amatrix()

    with tile.TileContext(nc) as tc:
        with (
            tc.tile_pool(name="work", bufs=1) as wpool,
            tc.tile_pool(name="psum", bufs=1, space="PSUM") as ppool,
        ):
            # ---- DMAs first: g (small, feeds the longest chain), then the
            # wt/ft pieces split across both HWDGE rings so the two queues
            # stream concurrently and each consumer waits only its piece
            pk_sb = wpool.tile([P, PK_LEN], bf16)
            for i, ename in enumerate(_PK_ENGS):
                eng = nc.sync if ename == "sync" else nc.scalar
                s, e = _PK_SPLITS[i], _PK_SPLITS[i + 1]
                eng.dma_start(out=pk_sb[:, s:e], in_=pk[:, s:e])

            # ---- on-chip constants (gpsimd) ----
            # esel[m, k, j] = A[j, k] for j < 16, 0 for j in 16..31.
            esel = wpool.tile([P, L, 32], bf16)
            nc.gpsimd.memset(esel, 0.0)
            esel_d = esel[:, 0:NDELTA, 0:L]
            # fill 1.0 on the delta diagonal (j == k)
            nc.gpsimd.affine_select(
                out=esel_d,
                in_=esel_d,
                compare_op=mybir.AluOpType.not_equal,
                fill=1.0,
                base=0,
                pattern=[[1, NDELTA], [-1, L]],
                channel_multiplier=0,
            )
            # delta planes also contribute -1 to count row 11
            nc.gpsimd.affine_select(
                out=esel_d,
                in_=esel_d,
                compare_op=mybir.AluOpType.not_equal,
                fill=-1.0,
                base=-NDELTA,
                pattern=[[0, NDELTA], [1, L]],
                channel_multiplier=0,
            )
            # ones plane column (needed by round 0)
            nc.gpsimd.memset(esel[:, 15, 11:12], float(A[11, 15]))
            # ones plane data + relu bias constants + identity
            planes = wpool.tile([P, L, BN], bf16)
            nc.gpsimd.memset(planes[:, 15, :], 1.0)
            rpb = wpool.tile([P, NRAMP], f32)
            for i in range(NRAMP):
                nc.gpsimd.memset(rpb[:, i : i + 1], -(NDELTA + 0.5 + i))
            ident = wpool.tile([P, P], bf16)
            make_identity(nc, ident)
            # ramp-plane selector columns (needed by round 3)
            for k in range(NDELTA, L - 1):
                for j in range(NDELTA, L):
                    v = float(A[j, k])
                    if v != 0.0:
                        nc.gpsimd.memset(esel[:, k, j : j + 1], v)

            # ---- ACT: table preload, then ramps as soon as G_bf exists ----
            act_warm = wpool.tile([P, 2], f32)
            nc.scalar.copy(out=act_warm[:, 0:1], in_=act_warm[:, 1:2])

            # ---- DVE stream (g arrives pre-cast to bf16 in the pk head) ----
            dummy = wpool.tile([P, BN], bf16)
            nc.vector.memset(dummy, 1.0)
            G_bf = pk_sb[:, G_OFF : G_OFF + BN]

            # ACT ramps (program order on scalar engine; wait on G_bf)
            for i in range(NRAMP):
                nc.scalar.activation(
                    out=planes[:, NDELTA + i, :],
                    in_=G_bf,
                    func=mybir.ActivationFunctionType.Relu,
                    bias=rpb[:, i : i + 1],
                    scale=1.0,
                )

            Wsum = wpool.tile([P, KC, D], bf16)

            def _wsum(c):
                nc.vector.tensor_tensor(
                    out=Wsum[:, c, :],
                    in0=pk_sb[:, _wt_off(c, 0) : _wt_off(c, 0) + D],
                    in1=pk_sb[:, _wt_off(c, 1) : _wt_off(c, 1) + D],
                    op=mybir.AluOpType.add,
                )
                sl = slice(c * P, (c + 1) * P)
                nc.vector.tensor_tensor(
                    out=Wsum[:, c, sl],
                    in0=Wsum[:, c, sl],
                    in1=ident,
                    op=mybir.AluOpType.add,
                )

            def _delta(l, eng):
                eng.tensor_scalar(
                    out=planes[:, l, :],
                    in0=G_bf,
                    scalar1=float(l),
                    scalar2=None,
                    op0=mybir.AluOpType.is_equal,
                )

            # DVE: the delta stream runs UNINTERRUPTED (it feeds the long
            # cnt->cntT->bias chain); Wsum adds follow in DMA arrival order:
            # c0 (sync, after g), c1 (scalar head), c2 (sync), c3 (scalar).
            # (gpsimd must NOT run elementwise here: its DVE-shared SBUF port
            # lock makes both engines ~25x slower — measured)
            for l in range(NDELTA):
                _delta(l, nc.vector)
            # arrival order: wt1 (scalar head) ~ wt0 (sync), then wt3, wt2
            for c in (1, 0, 3, 2):
                _wsum(c)

            # ---- PE stream ----
            psum_warm = ppool.tile([P, BN], f32, tag="warm", bufs=1)
            psum_cnt = ppool.tile([P, BN], f32, tag="cnt", bufs=1)
            psum_outs = [
                ppool.tile([P, D], f32, tag=f"out{b}", bufs=1, name=f"psum_out{b}")
                for b in range(BPC)
            ]

            for _ in range(FILLERS):
                nc.tensor.matmul(
                    out=psum_warm, lhsT=dummy[:, 0:P], rhs=dummy,
                    start=True, stop=True,
                )

            def cnt_mm(i):
                k = CNT_ORDER[i]
                s = i % 4
                nc.tensor.matmul(
                    out=psum_cnt[32 * s : 32 * s + 32, :],
                    lhsT=esel[:, k, :],
                    rhs=planes[:, k, :],
                    start=i < 4,
                    stop=i >= 12,
                    tile_position=(0, 32 * s),
                )

            def mains(c, start=False, stop=False):
                fo = _ft_off(c)
                for b in range(BPC):
                    nc.tensor.matmul(
                        out=psum_outs[b],
                        lhsT=pk_sb[:, fo + b * P : fo + (b + 1) * P],
                        rhs=Wsum[:, c, :],
                        start=start,
                        stop=stop,
                    )

            for i in range(16):
                cnt_mm(i)

            # single whole-tile copy: 4 slice copies self-serialize on ACT
            # with ~400ns sem gaps each (measured) and finish later
            cntT = wpool.tile([P, BN], bf16)
            nc.scalar.copy(out=cntT, in_=psum_cnt)

            # bridge filler keeps the PE HAM-warm between the cnt stream and
            # the dense tail
            nc.tensor.matmul(
                out=psum_warm, lhsT=dummy[:, 0:P], rhs=dummy,
                start=True, stop=True,
            )

            # tail: mains c1 (earliest Wsum), bias, mains c0, c3, then c2
            # (last to land) closing each output bank progressively
            cb4 = pk_sb[:, CB4_OFF : CB4_OFF + D]
            out_sb = wpool.tile([P, BPC, D], bf16)
            h = D // 2
            mains(1, start=True)
            for b in range(BPC):
                nc.tensor.matmul(
                    out=psum_outs[b],
                    lhsT=cntT[:, b * P : (b + 1) * P],
                    rhs=cb4,
                    start=False,
                    stop=False,
                )
            mains(0)
            mains(3)
            fo2 = _ft_off(2)
            for b in range(BPC):
                nc.tensor.matmul(
                    out=psum_outs[b],
                    lhsT=pk_sb[:, fo2 + b * P : fo2 + (b + 1) * P],
                    rhs=Wsum[:, 2, :],
                    start=False,
                    stop=True,
                )
                nc.vector.tensor_copy(out=out_sb[:, b, 0:h], in_=psum_outs[b][:, 0:h])
                nc.scalar.copy(out=out_sb[:, b, h:D], in_=psum_outs[b][:, h:D])
                # two paired output DMAs (fewer ~650ns HWDGE issues): banks
                # 0-1 on the sync ring, banks 2-3 on the scalar ring
                if b == 1:
                    nc.sync.dma_start(
                        out=out[0:2].rearrange("b n d -> n b d"),
                        in_=out_sb[:, 0:2, :],
                    )
                elif b == 3:
                    nc.scalar.dma_start(
                        out=out[2:4].rearrange("b n d -> n b d"),
                        in_=out_sb[:, 2:4, :],
                    )

    nc.compile()
    return nc


def _get_prog():
    if "v7" not in _prog_cache:
        _prog_cache["v7"] = _build()
    return _prog_cache["v7"]


def _shard_inputs(feature, graph, weights, bias):
    f = np.asarray(feature, dtype=np.float32)
    g8 = np.asarray(graph).astype(np.uint8)
    w = np.asarray(weights, dtype=np.float32)
    b16 = np.asarray(bias, dtype=np.float32).astype(BF16)

    # cb4[p] = bias[p % 32] if p % 32 < 16 else 0   (strip-replicated)
    cb4 = np.zeros((P, D), dtype=BF16)
    for s in range(4):
        cb4[32 * s : 32 * s + L] = b16

    # wt[p, c, d, o] = w[d, o, c*128+p]   (replicated across cores)
    wt = np.ascontiguousarray(
        w.transpose(2, 0, 1).reshape(KC, P, DIR, D).transpose(1, 0, 2, 3)
    ).astype(BF16)  # [p, c, d, o]

    in_maps = []
    for core in range(NC):
        sl = slice(core * BPC, (core + 1) * BPC)
        fc = f[sl]  # [BPC, N, D]
        ftc = np.ascontiguousarray(
            fc.transpose(2, 0, 1).reshape(KC, P, BN).transpose(1, 0, 2)
        ).astype(BF16)  # [p, c, bn]
        gc = np.ascontiguousarray(g8[sl].transpose(2, 0, 1).reshape(P, BN))
        pk = np.empty((P, PK_LEN), dtype=BF16)
        pk[:, G_OFF : G_OFF + BN] = gc.astype(BF16)
        pk[:, CB4_OFF : CB4_OFF + D] = cb4
        for c in range(KC):
            pk[:, _ft_off(c) : _ft_off(c) + BN] = ftc[:, c, :]
            pk[:, _wt_off(c, 0) : _wt_off(c, 1) + D] = wt[:, c].reshape(P, DIR * D)
        in_maps.append({"pk": pk})
    return in_maps


def _run(feature, graph, weights, bias, trace=False):
    from concourse.bass_utils import run_bass_kernel_spmd

    in_maps = _shard_inputs(feature, graph, weights, bias)
    nc = _get_prog()
    res = run_bass_kernel_spmd(nc, in_maps, core_ids=list(range(NC)), trace=trace)
    out = np.concatenate(
        [r["out"].astype(np.float32) for r in res.results], axis=0
    )
    return out, res


def kernel(feature, graph, weights, bias):
    out, _ = _run(feature, graph, weights, bias, trace=False)
    return out
